# revision 46
# baseline (speedup 1.0000x reference)
"""Trainium2 Bass kernel for FConv2d (FFT conv module).

out = irfftn( rfftn(x, axes=(c,h,w)) * rfftn(pad(weight)) )[:, :, ::4] reshaped.

Strategy (data-parallel over batch, 4 per core x 8 cores):
  S1  channel DFT (c=128) as matmuls; also transposes hw chunks onto partitions
  S2  joint 2D spatial DFT (contract hw=1024 via PSUM-accumulated matmuls);
      the complex combine is folded into the accumulation via a negated X1i
  MUL elementwise complex multiply with host-precomputed folded W-hat   (DVE)
  I1  fold(128->32) + subsampled channel IDFT via paired complex matmuls
  I2  joint 2D spatial inverse + real-part extraction via paired matmuls

All matmuls run as float32r (full-rate fp32 mode on TRN2 PE).
Constants (DFT factor matrices, W-hat) are computed host-side in numpy and
fed as replicated ExternalInputs.
"""
import sys
import numpy as np

for _p in ("/opt/trn_rl_repo", "/root/.axon_site/_ro/trn_rl_repo"):
    if _p not in sys.path:
        sys.path.insert(0, _p)

import concourse.bacc as bacc
import concourse.bass as bass
import concourse.mybir as mybir
import concourse.tile as tile
from concourse.bass_utils import run_bass_kernel_spmd

F32 = mybir.dt.float32
F32R = mybir.dt.float32r

B = 32          # full batch
B_LOC = 4       # per core
N_CORES = 8
CIN = 128
L = 32
NFIL = 8        # num filters n
NF = 544        # stored spatial freqs (32 * 17)
NFP = 640       # padded: 5 chunks of 128


# ----------------------------------------------------------------- constants
def build_constants(weight):
    c = np.arange(128)
    k = np.arange(128)
    ang = 2 * np.pi * np.outer(c, k) / 128.0
    fc_pack = np.concatenate([np.cos(ang), -np.sin(ang)], axis=1).astype(np.float32)

    f = np.arange(NFP)
    p = np.where(f < NF, f // 17, 0)
    q = np.where(f < NF, f % 17, 0)
    valid = (f < NF).astype(np.float64)

    # f2d [128, 8, 5, 384]: cols of block fb: [cos | -sin | +sin]
    f2d = np.zeros((128, 8, 5, 384), dtype=np.float32)
    hw_p = np.arange(128)
    for t in range(8):
        h = 4 * t + hw_p // 32
        w = hw_p % 32
        for fb in range(5):
            sl = slice(fb * 128, (fb + 1) * 128)
            ang2 = 2 * np.pi * (np.outer(h, p[sl]) + np.outer(w, q[sl])) / 32.0
            f2d[:, t, fb, 0:128] = np.cos(ang2) * valid[sl]
            f2d[:, t, fb, 128:256] = -np.sin(ang2) * valid[sl]
            f2d[:, t, fb, 256:384] = np.sin(ang2) * valid[sl]

    kk = np.arange(128)
    j = np.arange(32)
    ang3 = 2 * np.pi * np.outer(kk, j) / 32.0
    er = np.cos(ang3)
    ei = np.sin(ang3)
    epack1 = np.concatenate([er, ei], axis=1).astype(np.float32)
    epack2 = np.concatenate([-ei, er], axis=1).astype(np.float32)

    k2d = np.zeros((128, 5, 2, 1024), dtype=np.float32)
    yz = np.arange(1024)
    y = yz // 32
    z = yz % 32
    for fc in range(5):
        sl = slice(fc * 128, (fc + 1) * 128)
        ang4 = 2 * np.pi * (np.outer(p[sl], y) + np.outer(q[sl], z)) / 32.0
        k2d[:, fc, 0, :] = np.cos(ang4) * valid[sl][:, None]
        k2d[:, fc, 1, :] = -np.sin(ang4) * valid[sl][:, None]

    w_hat = np.fft.rfftn(weight.astype(np.float64), s=(CIN, L, L), axes=(1, 2, 3))
    alpha = np.full(17, 2.0)
    alpha[0] = 1.0
    alpha[16] = 1.0
    w_hat = w_hat * alpha[None, None, None, :] / (128.0 * 32.0 * 32.0)
    wr = np.zeros((128, NFIL, NFP), dtype=np.float32)
    wi = np.zeros((128, NFIL, NFP), dtype=np.float32)
    wr[:, :, :NF] = np.transpose(w_hat.real, (1, 0, 2, 3)).reshape(128, NFIL, NF)
    wi[:, :, :NF] = np.transpose(w_hat.imag, (1, 0, 2, 3)).reshape(128, NFIL, NF)
    return {
        "fc_pack": fc_pack,
        "f2d": f2d,
        "epack1": epack1,
        "epack2": epack2,
        "k2d": k2d,
        "wr": wr,
        "wi": wi,
    }


# ----------------------------------------------------------------- program
def build_program(dbg=False):
    nc = bacc.Bacc("TRN2", target_bir_lowering=False, debug=False)
    x_d = nc.dram_tensor("x", [B_LOC, 128, 32, 32], F32R, kind="ExternalInput")
    fc_d = nc.dram_tensor("fc_pack", [128, 256], F32R, kind="ExternalInput")
    f2d_d = nc.dram_tensor("f2d", [128, 8, 5, 384], F32R, kind="ExternalInput")
    e1_d = nc.dram_tensor("epack1", [128, 64], F32R, kind="ExternalInput")
    e2_d = nc.dram_tensor("epack2", [128, 64], F32R, kind="ExternalInput")
    k2d_d = nc.dram_tensor("k2d", [128, 5, 2, 1024], F32R, kind="ExternalInput")
    wr_d = nc.dram_tensor("wr", [128, NFIL, NFP], F32R, kind="ExternalInput")
    wi_d = nc.dram_tensor("wi", [128, NFIL, NFP], F32R, kind="ExternalInput")
    out_d = nc.dram_tensor("out", [B_LOC, 256, 32, 32], F32, kind="ExternalOutput")
    if dbg:
        dbg_x1 = nc.dram_tensor("dbg_x1", [128, 8, 2, 128], F32R, kind="ExternalOutput")
        dbg_xf = nc.dram_tensor("dbg_xf", [128, 2, NFP], F32R, kind="ExternalOutput")
        dbg_a = nc.dram_tensor("dbg_a", [128, 5, 2, NFIL, 32], F32R, kind="ExternalOutput")

    with tile.TileContext(nc) as tc:
        with (
            tc.tile_pool(name="consts", bufs=1) as cpool,
            tc.tile_pool(name="xin", bufs=1) as xpool,
            tc.tile_pool(name="x1", bufs=2) as x1pool,
            tc.tile_pool(name="xf", bufs=2) as xfpool,
            tc.tile_pool(name="z", bufs=2) as zpool,
            tc.tile_pool(name="a", bufs=1) as apool,
            tc.tile_pool(name="o", bufs=2) as opool,
            tc.tile_pool(name="ps1", bufs=1, space="PSUM") as ps1pool,
            tc.tile_pool(name="ps2", bufs=3, space="PSUM") as ps2pool,
            tc.tile_pool(name="psi1", bufs=2, space="PSUM") as psi1pool,
            tc.tile_pool(name="psi2", bufs=2, space="PSUM") as psi2pool,
        ):
            # ---- load constants; ordered + chunked so early compute
            # stages never wait on late-needed constants.
            fc_sb = cpool.tile([128, 256], F32R)
            nc.sync.dma_start(out=fc_sb[:], in_=fc_d[:])
            e1_sb = cpool.tile([128, 64], F32R)
            nc.sync.dma_start(out=e1_sb[:], in_=e1_d[:])
            e2_sb = cpool.tile([128, 64], F32R)
            nc.sync.dma_start(out=e2_sb[:], in_=e2_d[:])
            f2d_sb = cpool.tile([128, 8, 5, 384], F32R)
            for t in range(8):
                nc.sync.dma_start(out=f2d_sb[:, t], in_=f2d_d[:, t])
            wr_sb = cpool.tile([128, NFIL, NFP], F32R)
            nc.sync.dma_start(out=wr_sb[:], in_=wr_d[:])
            wi_sb = cpool.tile([128, NFIL, NFP], F32R)
            nc.sync.dma_start(out=wi_sb[:], in_=wi_d[:])
            k2d_sb = cpool.tile([128, 5, 2, 1024], F32R)
            for fc in range(5):
                nc.sync.dma_start(out=k2d_sb[:, fc], in_=k2d_d[:, fc])

            for b in range(B_LOC):
                # ---- load x[b]: [c=128, h, w]
                xt = xpool.tile([128, 32, 32], F32R, tag="xt")
                nc.scalar.dma_start(out=xt[:], in_=x_d[b])

                # ---- S1: channel DFT; x1 slots: 0=X1r, 1=X1i
                x1 = x1pool.tile([128, 8, 2, 128], F32R, tag="x1")
                for t in range(8):
                    ps = ps1pool.tile([128, 256], F32, tag="ps1")
                    nc.tensor.matmul(
                        ps[:], xt[:, 4 * t:4 * t + 4, :], fc_sb[:],
                        start=True, stop=True,
                    )
                    nc.vector.tensor_copy(x1[:, t, 0, :], ps[:, 0:128])
                    nc.scalar.copy(x1[:, t, 1, :], ps[:, 128:256])

                if dbg and b == 0:
                    nc.sync.dma_start(out=dbg_x1[:], in_=x1[:])

                # ---- S2: joint 2D DFT, complex combine via PSUM accumulation.
                # psum block fb: cols [Xr(128) | Xi(128)]
                # Xr = X1r@cos + X1i@sin ;  Xi = X1i@cos - X1r@sin
                s2ps = [
                    ps2pool.tile([128, 512], F32, tag="ps2", name=f"s2ps{b}_{g}")
                    for g in range(3)
                ]

                def s2slot(fb):
                    return s2ps[fb // 2][:, (fb % 2) * 256:(fb % 2) * 256 + 256]

                # One accumulation group per PSUM bank: the A-pass spans the
                # whole bank (fb-pair) and carries the only start=True.
                # Xr = X1r@cos + X1i@sin ; Xi = X1i@cos - X1r@sin.
                # B-passes are fb-paired (2D APs) so N=256 keeps fp32r at
                # full rate.
                for t in range(8):
                    # A-passes (lhsT = X1r): [cos | -sin] per fb
                    for g in range(3):
                        slot = s2ps[g]
                        if g < 2:
                            nc.tensor.matmul(
                                slot[:, 0:512], x1[:, t, 0, :],
                                f2d_sb[:, t, 2 * g:2 * g + 2, 0:256],
                                start=(t == 0), stop=False,
                                skip_group_check=True,
                            )
                        else:
                            nc.tensor.matmul(
                                slot[:, 0:256], x1[:, t, 0, :],
                                f2d_sb[:, t, 4, 0:256],
                                start=(t == 0), stop=False,
                                skip_group_check=True,
                            )
                    # B-passes (lhsT = X1i): +sin -> Xr half, cos -> Xi half
                    for g in range(3):
                        slot = s2ps[g]
                        if g < 2:
                            sv = slot[:].rearrange("p (a c) -> p a c", a=2)
                            pair = slice(2 * g, 2 * g + 2)
                            nc.tensor.matmul(
                                sv[:, :, 0:128], x1[:, t, 1, :],
                                f2d_sb[:, t, pair, 256:384],
                                start=False, stop=(t == 7),
                                skip_group_check=True,
                            )
                            nc.tensor.matmul(
                                sv[:, :, 128:256], x1[:, t, 1, :],
                                f2d_sb[:, t, pair, 0:128],
                                start=False, stop=(t == 7),
                                skip_group_check=True,
                            )
                        else:
                            nc.tensor.matmul(
                                slot[:, 0:128], x1[:, t, 1, :],
                                f2d_sb[:, t, 4, 256:384],
                                start=False, stop=(t == 7),
                                skip_group_check=True,
                            )
                            nc.tensor.matmul(
                                slot[:, 128:256], x1[:, t, 1, :],
                                f2d_sb[:, t, 4, 0:128],
                                start=False, stop=(t == 7),
                                skip_group_check=True,
                            )

                # copy PSUM -> SBUF, de-interleaving to contiguous
                # xr_c / xi_c [128, 640] (f-major) for fast 1D DVE ops
                xr_c = xfpool.tile([128, NFP], F32R, tag="xr")
                xi_c = xfpool.tile([128, NFP], F32R, tag="xi")
                for g in range(3):
                    slot = s2ps[g]
                    if g < 2:
                        sv = slot[:].rearrange("p (a c) -> p a c", a=2)
                        nc.vector.tensor_copy(
                            xr_c[:, g * 256:(g + 1) * 256], sv[:, :, 0:128])
                        nc.vector.tensor_copy(
                            xi_c[:, g * 256:(g + 1) * 256], sv[:, :, 128:256])
                    else:
                        nc.vector.tensor_copy(xr_c[:, 512:640], slot[:, 0:128])
                        nc.vector.tensor_copy(xi_c[:, 512:640], slot[:, 128:256])

                if dbg and b == 0:
                    nc.sync.dma_start(out=dbg_xf[:, 0], in_=xr_c[:])
                    nc.sync.dma_start(out=dbg_xf[:, 1], in_=xi_c[:])

                # ---- per-n: complex multiply (DVE) + I1 matmuls
                a_sb = apool.tile([128, 5, 2, NFIL, 32], F32R, tag="a")
                for n in range(8):
                    zr = zpool.tile([128, NFP], F32R, tag="zr", bufs=3)
                    zi = zpool.tile([128, NFP], F32R, tag="zi", bufs=3)
                    t1 = zpool.tile([128, NFP], F32R, tag="t1", bufs=1)
                    t2 = zpool.tile([128, NFP], F32R, tag="t2", bufs=1)
                    nc.vector.tensor_mul(t1[:], xr_c[:], wr_sb[:, n, :])
                    nc.vector.tensor_mul(t2[:], xi_c[:], wi_sb[:, n, :])
                    nc.vector.tensor_sub(zr[:], t1[:], t2[:])
                    nc.vector.tensor_mul(t1[:], xr_c[:], wi_sb[:, n, :])
                    nc.vector.tensor_mul(t2[:], xi_c[:], wr_sb[:, n, :])
                    nc.vector.tensor_add(zi[:], t1[:], t2[:])

                    ips = psi1pool.tile([128, 320], F32, tag="psi1")
                    for fc in range(5):
                        col = slice(fc * 64, (fc + 1) * 64)
                        zsl = slice(fc * 128, (fc + 1) * 128)
                        nc.tensor.matmul(
                            ips[:, col], zr[:, zsl], e1_sb[:],
                            start=True, stop=False,
                        )
                        nc.tensor.matmul(
                            ips[:, col], zi[:, zsl], e2_sb[:],
                            start=False, stop=True,
                        )
                    # scatter [fc, comp, j] cols of ips into a_sb[:, fc, comp, n, :]
                    nc.scalar.copy(
                        a_sb[:, :, :, n, :],
                        ips[:].rearrange("p (fc c j) -> p fc c j", fc=5, c=2),
                    )

                if dbg and b == 0:
                    nc.sync.dma_start(out=dbg_a[:], in_=a_sb[:])

                # ---- I2: joint 2D inverse + Re extraction
                for mh in range(2):
                    for nzc in range(2):
                        ops = psi2pool.tile([128, 512], F32, tag="psi2")
                        for fc in range(5):
                            for comp in range(2):
                                lhsT = a_sb[:, fc, comp,
                                            mh * 4:(mh + 1) * 4, :]
                                rhs = k2d_sb[:, fc, comp,
                                             nzc * 512:(nzc + 1) * 512]
                                nc.tensor.matmul(
                                    ops[:], lhsT, rhs,
                                    start=(fc == 0 and comp == 0),
                                    stop=(fc == 4 and comp == 1),
                                )
                        o_sb = opool.tile([128, 512], F32, tag="o")
                        nc.vector.tensor_copy(o_sb[:], ops[:])
                        dst = out_d[b, mh * 128:(mh + 1) * 128].rearrange(
                            "c h w -> c (h w)")[:, nzc * 512:(nzc + 1) * 512]
                        nc.scalar.dma_start(out=dst, in_=o_sb[:])
    nc.compile()
    return nc


_CACHE = {}


def kernel(x, weight):
    x = np.ascontiguousarray(np.asarray(x, dtype=np.float32))
    weight = np.asarray(weight, dtype=np.float32)
    consts = build_constants(weight)
    if "nc" not in _CACHE:
        _CACHE["nc"] = build_program()
    nc = _CACHE["nc"]
    in_maps = []
    for i in range(N_CORES):
        m = {"x": x[i * B_LOC:(i + 1) * B_LOC]}
        m.update(consts)
        in_maps.append(m)
    res = run_bass_kernel_spmd(nc, in_maps, core_ids=list(range(N_CORES)))
    out = np.concatenate([r["out"] for r in res.results], axis=0)
    return out


if __name__ == "__main__":
    import jax

    sys.path.insert(0, "/root/problem")
    from reference import setup_inputs, reference

    with jax.default_device(jax.devices("cpu")[0]):
        inputs = setup_inputs()
        inputs = {k: np.asarray(v) for k, v in inputs.items()}
        expected = np.asarray(reference(**inputs))
    actual = kernel(**inputs)
    err = np.linalg.norm(actual - expected) / np.linalg.norm(expected)
    print("Relative error:", err)


# revision 48
# speedup vs baseline: 1.0515x; 1.0515x over previous
"""Trainium2 Bass kernel for FConv2d (FFT conv module).

out = irfftn( rfftn(x, axes=(c,h,w)) * rfftn(pad(weight)) )[:, :, ::4] reshaped.

Strategy (data-parallel over batch, 4 per core x 8 cores):
  S1  channel DFT (c=128) as matmuls; also transposes hw chunks onto partitions
  S2  joint 2D spatial DFT (contract hw=1024 via PSUM-accumulated matmuls);
      the complex combine is folded into the accumulation via a negated X1i
  MUL elementwise complex multiply with host-precomputed folded W-hat   (DVE)
  I1  fold(128->32) + subsampled channel IDFT via paired complex matmuls
  I2  joint 2D spatial inverse + real-part extraction via paired matmuls

All matmuls run as float32r (full-rate fp32 mode on TRN2 PE).
Constants (DFT factor matrices, W-hat) are computed host-side in numpy and
fed as replicated ExternalInputs.
"""
import sys
import numpy as np

for _p in ("/opt/trn_rl_repo", "/root/.axon_site/_ro/trn_rl_repo"):
    if _p not in sys.path:
        sys.path.insert(0, _p)

import concourse.bacc as bacc
import concourse.bass as bass
import concourse.mybir as mybir
import concourse.tile as tile
from concourse.bass_utils import run_bass_kernel_spmd

F32 = mybir.dt.float32
F32R = mybir.dt.float32r

B = 32          # full batch
B_LOC = 4       # per core
N_CORES = 8
CIN = 128
L = 32
NFIL = 8        # num filters n
NF = 544        # stored spatial freqs (32 * 17)
NFP = 640       # padded: 5 chunks of 128


# ----------------------------------------------------------------- constants
def build_constants(weight):
    c = np.arange(128)
    k = np.arange(128)
    ang = 2 * np.pi * np.outer(c, k) / 128.0
    fc_pack = np.concatenate([np.cos(ang), -np.sin(ang)], axis=1).astype(np.float32)

    f = np.arange(NFP)
    p = np.where(f < NF, f // 17, 0)
    q = np.where(f < NF, f % 17, 0)
    valid = (f < NF).astype(np.float64)

    # f2d [128, 8, 5, 384]: cols of block fb: [cos | -sin | +sin]
    f2d = np.zeros((128, 8, 5, 384), dtype=np.float32)
    hw_p = np.arange(128)
    for t in range(8):
        h = 4 * t + hw_p // 32
        w = hw_p % 32
        for fb in range(5):
            sl = slice(fb * 128, (fb + 1) * 128)
            ang2 = 2 * np.pi * (np.outer(h, p[sl]) + np.outer(w, q[sl])) / 32.0
            f2d[:, t, fb, 0:128] = np.cos(ang2) * valid[sl]
            f2d[:, t, fb, 128:256] = -np.sin(ang2) * valid[sl]
            f2d[:, t, fb, 256:384] = np.sin(ang2) * valid[sl]

    kk = np.arange(128)
    j = np.arange(32)
    ang3 = 2 * np.pi * np.outer(kk, j) / 32.0
    er = np.cos(ang3)
    ei = np.sin(ang3)
    epack1 = np.concatenate([er, ei], axis=1).astype(np.float32)
    epack2 = np.concatenate([-ei, er], axis=1).astype(np.float32)

    k2d = np.zeros((128, 5, 2, 1024), dtype=np.float32)
    yz = np.arange(1024)
    y = yz // 32
    z = yz % 32
    for fc in range(5):
        sl = slice(fc * 128, (fc + 1) * 128)
        ang4 = 2 * np.pi * (np.outer(p[sl], y) + np.outer(q[sl], z)) / 32.0
        k2d[:, fc, 0, :] = np.cos(ang4) * valid[sl][:, None]
        k2d[:, fc, 1, :] = -np.sin(ang4) * valid[sl][:, None]

    w_hat = np.fft.rfftn(weight.astype(np.float64), s=(CIN, L, L), axes=(1, 2, 3))
    alpha = np.full(17, 2.0)
    alpha[0] = 1.0
    alpha[16] = 1.0
    w_hat = w_hat * alpha[None, None, None, :] / (128.0 * 32.0 * 32.0)
    wr = np.zeros((128, NFIL, NFP), dtype=np.float32)
    wi = np.zeros((128, NFIL, NFP), dtype=np.float32)
    wr[:, :, :NF] = np.transpose(w_hat.real, (1, 0, 2, 3)).reshape(128, NFIL, NF)
    wi[:, :, :NF] = np.transpose(w_hat.imag, (1, 0, 2, 3)).reshape(128, NFIL, NF)
    return {
        "fc_pack": fc_pack,
        "f2d": f2d,
        "epack1": epack1,
        "epack2": epack2,
        "k2d": k2d,
        "wr": wr,
        "wi": wi,
    }


# ----------------------------------------------------------------- program
def build_program(dbg=False):
    nc = bacc.Bacc("TRN2", target_bir_lowering=False, debug=False)
    x_d = nc.dram_tensor("x", [B_LOC, 128, 32, 32], F32R, kind="ExternalInput")
    fc_d = nc.dram_tensor("fc_pack", [128, 256], F32R, kind="ExternalInput")
    f2d_d = nc.dram_tensor("f2d", [128, 8, 5, 384], F32R, kind="ExternalInput")
    e1_d = nc.dram_tensor("epack1", [128, 64], F32R, kind="ExternalInput")
    e2_d = nc.dram_tensor("epack2", [128, 64], F32R, kind="ExternalInput")
    k2d_d = nc.dram_tensor("k2d", [128, 5, 2, 1024], F32R, kind="ExternalInput")
    wr_d = nc.dram_tensor("wr", [128, NFIL, NFP], F32R, kind="ExternalInput")
    wi_d = nc.dram_tensor("wi", [128, NFIL, NFP], F32R, kind="ExternalInput")
    out_d = nc.dram_tensor("out", [B_LOC, 256, 32, 32], F32, kind="ExternalOutput")
    if dbg:
        dbg_x1 = nc.dram_tensor("dbg_x1", [128, 8, 2, 128], F32R, kind="ExternalOutput")
        dbg_xf = nc.dram_tensor("dbg_xf", [128, 2, NFP], F32R, kind="ExternalOutput")
        dbg_a = nc.dram_tensor("dbg_a", [128, 5, 2, NFIL, 32], F32R, kind="ExternalOutput")

    with tile.TileContext(nc) as tc:
        with (
            tc.tile_pool(name="consts", bufs=1) as cpool,
            tc.tile_pool(name="xin", bufs=2) as xpool,
            tc.tile_pool(name="x1", bufs=2) as x1pool,
            tc.tile_pool(name="xf", bufs=2) as xfpool,
            tc.tile_pool(name="z", bufs=2) as zpool,
            tc.tile_pool(name="a", bufs=1) as apool,
            tc.tile_pool(name="o", bufs=2) as opool,
            tc.tile_pool(name="ps1", bufs=1, space="PSUM") as ps1pool,
            tc.tile_pool(name="ps2", bufs=3, space="PSUM") as ps2pool,
            tc.tile_pool(name="psi1", bufs=2, space="PSUM") as psi1pool,
            tc.tile_pool(name="psi2", bufs=2, space="PSUM") as psi2pool,
        ):
            # ---- load constants; ordered + chunked so early compute
            # stages never wait on late-needed constants.
            fc_sb = cpool.tile([128, 256], F32R)
            nc.sync.dma_start(out=fc_sb[:], in_=fc_d[:])
            e1_sb = cpool.tile([128, 64], F32R)
            nc.sync.dma_start(out=e1_sb[:], in_=e1_d[:])
            e2_sb = cpool.tile([128, 64], F32R)
            nc.sync.dma_start(out=e2_sb[:], in_=e2_d[:])
            f2d_sb = cpool.tile([128, 8, 5, 384], F32R)
            for t in range(8):
                nc.sync.dma_start(out=f2d_sb[:, t], in_=f2d_d[:, t])
            wr_sb = cpool.tile([128, NFIL, NFP], F32R)
            nc.sync.dma_start(out=wr_sb[:], in_=wr_d[:])
            wi_sb = cpool.tile([128, NFIL, NFP], F32R)
            nc.sync.dma_start(out=wi_sb[:], in_=wi_d[:])
            k2d_sb = cpool.tile([128, 5, 2, 1024], F32R)
            for fc in range(5):
                nc.sync.dma_start(out=k2d_sb[:, fc], in_=k2d_d[:, fc])

            for b in range(B_LOC):
                # ---- load x[b]: [c=128, h, w]
                xt = xpool.tile([128, 32, 32], F32R, tag="xt")
                nc.scalar.dma_start(out=xt[:], in_=x_d[b])

                # ---- S1: channel DFT; x1 slots: 0=X1r, 1=X1i
                x1 = x1pool.tile([128, 8, 2, 128], F32R, tag="x1")
                for t in range(8):
                    ps = ps1pool.tile([128, 256], F32, tag="ps1")
                    nc.tensor.matmul(
                        ps[:], xt[:, 4 * t:4 * t + 4, :], fc_sb[:],
                        start=True, stop=True,
                    )
                    nc.vector.tensor_copy(x1[:, t, 0, :], ps[:, 0:128])
                    nc.scalar.copy(x1[:, t, 1, :], ps[:, 128:256])

                if dbg and b == 0:
                    nc.sync.dma_start(out=dbg_x1[:], in_=x1[:])

                # ---- S2: joint 2D DFT, complex combine via PSUM accumulation.
                # psum block fb: cols [Xr(128) | Xi(128)]
                # Xr = X1r@cos + X1i@sin ;  Xi = X1i@cos - X1r@sin
                s2ps = [
                    ps2pool.tile([128, 512], F32, tag="ps2", name=f"s2ps{b}_{g}")
                    for g in range(3)
                ]

                def s2slot(fb):
                    return s2ps[fb // 2][:, (fb % 2) * 256:(fb % 2) * 256 + 256]

                # One accumulation group per PSUM bank: the A-pass spans the
                # whole bank (fb-pair) and carries the only start=True.
                # Xr = X1r@cos + X1i@sin ; Xi = X1i@cos - X1r@sin.
                # B-passes are fb-paired (2D APs) so N=256 keeps fp32r at
                # full rate.
                for t in range(8):
                    # A-passes (lhsT = X1r): [cos | -sin] per fb
                    for g in range(3):
                        slot = s2ps[g]
                        if g < 2:
                            nc.tensor.matmul(
                                slot[:, 0:512], x1[:, t, 0, :],
                                f2d_sb[:, t, 2 * g:2 * g + 2, 0:256],
                                start=(t == 0), stop=False,
                                skip_group_check=True,
                            )
                        else:
                            nc.tensor.matmul(
                                slot[:, 0:256], x1[:, t, 0, :],
                                f2d_sb[:, t, 4, 0:256],
                                start=(t == 0), stop=False,
                                skip_group_check=True,
                            )
                    # B-passes (lhsT = X1i): +sin -> Xr half, cos -> Xi half
                    for g in range(3):
                        slot = s2ps[g]
                        if g < 2:
                            sv = slot[:].rearrange("p (a c) -> p a c", a=2)
                            pair = slice(2 * g, 2 * g + 2)
                            nc.tensor.matmul(
                                sv[:, :, 0:128], x1[:, t, 1, :],
                                f2d_sb[:, t, pair, 256:384],
                                start=False, stop=(t == 7),
                                skip_group_check=True,
                            )
                            nc.tensor.matmul(
                                sv[:, :, 128:256], x1[:, t, 1, :],
                                f2d_sb[:, t, pair, 0:128],
                                start=False, stop=(t == 7),
                                skip_group_check=True,
                            )
                        else:
                            nc.tensor.matmul(
                                slot[:, 0:128], x1[:, t, 1, :],
                                f2d_sb[:, t, 4, 256:384],
                                start=False, stop=(t == 7),
                                skip_group_check=True,
                            )
                            nc.tensor.matmul(
                                slot[:, 128:256], x1[:, t, 1, :],
                                f2d_sb[:, t, 4, 0:128],
                                start=False, stop=(t == 7),
                                skip_group_check=True,
                            )

                # copy PSUM -> SBUF, de-interleaving to contiguous
                # xr_c / xi_c [128, 640] (f-major) for fast 1D DVE ops
                xr_c = xfpool.tile([128, NFP], F32R, tag="xr")
                xi_c = xfpool.tile([128, NFP], F32R, tag="xi")
                for g in range(3):
                    slot = s2ps[g]
                    if g < 2:
                        sv = slot[:].rearrange("p (a c) -> p a c", a=2)
                        nc.vector.tensor_copy(
                            xr_c[:, g * 256:(g + 1) * 256], sv[:, :, 0:128])
                        nc.vector.tensor_copy(
                            xi_c[:, g * 256:(g + 1) * 256], sv[:, :, 128:256])
                    else:
                        nc.vector.tensor_copy(xr_c[:, 512:640], slot[:, 0:128])
                        nc.vector.tensor_copy(xi_c[:, 512:640], slot[:, 128:256])

                if dbg and b == 0:
                    nc.sync.dma_start(out=dbg_xf[:, 0], in_=xr_c[:])
                    nc.sync.dma_start(out=dbg_xf[:, 1], in_=xi_c[:])

                # ---- per-n: complex multiply (DVE) + I1 matmuls
                a_sb = apool.tile([128, 5, 2, NFIL, 32], F32R, tag="a")
                for n in range(8):
                    zr = zpool.tile([128, NFP], F32R, tag="zr")
                    zi = zpool.tile([128, NFP], F32R, tag="zi")
                    t1 = zpool.tile([128, NFP], F32R, tag="t1", bufs=1)
                    t2 = zpool.tile([128, NFP], F32R, tag="t2", bufs=1)
                    nc.vector.tensor_mul(t1[:], xr_c[:], wr_sb[:, n, :])
                    nc.vector.tensor_mul(t2[:], xi_c[:], wi_sb[:, n, :])
                    nc.vector.tensor_sub(zr[:], t1[:], t2[:])
                    nc.vector.tensor_mul(t1[:], xr_c[:], wi_sb[:, n, :])
                    nc.vector.tensor_mul(t2[:], xi_c[:], wr_sb[:, n, :])
                    nc.vector.tensor_add(zi[:], t1[:], t2[:])

                    ips = psi1pool.tile([128, 320], F32, tag="psi1")
                    for fc in range(5):
                        col = slice(fc * 64, (fc + 1) * 64)
                        zsl = slice(fc * 128, (fc + 1) * 128)
                        nc.tensor.matmul(
                            ips[:, col], zr[:, zsl], e1_sb[:],
                            start=True, stop=False,
                        )
                        nc.tensor.matmul(
                            ips[:, col], zi[:, zsl], e2_sb[:],
                            start=False, stop=True,
                        )
                    # scatter [fc, comp, j] cols of ips into a_sb[:, fc, comp, n, :]
                    nc.scalar.copy(
                        a_sb[:, :, :, n, :],
                        ips[:].rearrange("p (fc c j) -> p fc c j", fc=5, c=2),
                    )

                if dbg and b == 0:
                    nc.sync.dma_start(out=dbg_a[:], in_=a_sb[:])

                # ---- I2: joint 2D inverse + Re extraction
                for mh in range(2):
                    for nzc in range(2):
                        ops = psi2pool.tile([128, 512], F32, tag="psi2")
                        for fc in range(5):
                            for comp in range(2):
                                lhsT = a_sb[:, fc, comp,
                                            mh * 4:(mh + 1) * 4, :]
                                rhs = k2d_sb[:, fc, comp,
                                             nzc * 512:(nzc + 1) * 512]
                                nc.tensor.matmul(
                                    ops[:], lhsT, rhs,
                                    start=(fc == 0 and comp == 0),
                                    stop=(fc == 4 and comp == 1),
                                )
                        o_sb = opool.tile([128, 512], F32, tag="o")
                        nc.vector.tensor_copy(o_sb[:], ops[:])
                        dst = out_d[b, mh * 128:(mh + 1) * 128].rearrange(
                            "c h w -> c (h w)")[:, nzc * 512:(nzc + 1) * 512]
                        nc.scalar.dma_start(out=dst, in_=o_sb[:])
    nc.compile()
    return nc


_CACHE = {}


def kernel(x, weight):
    x = np.ascontiguousarray(np.asarray(x, dtype=np.float32))
    weight = np.asarray(weight, dtype=np.float32)
    consts = build_constants(weight)
    if "nc" not in _CACHE:
        _CACHE["nc"] = build_program()
    nc = _CACHE["nc"]
    in_maps = []
    for i in range(N_CORES):
        m = {"x": x[i * B_LOC:(i + 1) * B_LOC]}
        m.update(consts)
        in_maps.append(m)
    res = run_bass_kernel_spmd(nc, in_maps, core_ids=list(range(N_CORES)))
    out = np.concatenate([r["out"] for r in res.results], axis=0)
    return out


if __name__ == "__main__":
    import jax

    sys.path.insert(0, "/root/problem")
    from reference import setup_inputs, reference

    with jax.default_device(jax.devices("cpu")[0]):
        inputs = setup_inputs()
        inputs = {k: np.asarray(v) for k, v in inputs.items()}
        expected = np.asarray(reference(**inputs))
    actual = kernel(**inputs)
    err = np.linalg.norm(actual - expected) / np.linalg.norm(expected)
    print("Relative error:", err)


# revision 51
# speedup vs baseline: 1.0588x; 1.0070x over previous
"""Trainium2 Bass kernel for FConv2d (FFT conv module).

out = irfftn( rfftn(x, axes=(c,h,w)) * rfftn(pad(weight)) )[:, :, ::4] reshaped.

Strategy (data-parallel over batch, 4 per core x 8 cores):
  S1  channel DFT (c=128) as matmuls; also transposes hw chunks onto partitions
  S2  joint 2D spatial DFT (contract hw=1024 via PSUM-accumulated matmuls);
      the complex combine is folded into the accumulation via a negated X1i
  MUL elementwise complex multiply with host-precomputed folded W-hat   (DVE)
  I1  fold(128->32) + subsampled channel IDFT via paired complex matmuls
  I2  joint 2D spatial inverse + real-part extraction via paired matmuls

All matmuls run as float32r (full-rate fp32 mode on TRN2 PE).
Constants (DFT factor matrices, W-hat) are computed host-side in numpy and
fed as replicated ExternalInputs.
"""
import sys
import numpy as np

for _p in ("/opt/trn_rl_repo", "/root/.axon_site/_ro/trn_rl_repo"):
    if _p not in sys.path:
        sys.path.insert(0, _p)

import concourse.bacc as bacc
import concourse.bass as bass
import concourse.mybir as mybir
import concourse.tile as tile
from concourse.bass_utils import run_bass_kernel_spmd

F32 = mybir.dt.float32
F32R = mybir.dt.float32r

B = 32          # full batch
B_LOC = 4       # per core
N_CORES = 8
CIN = 128
L = 32
NFIL = 8        # num filters n
NF = 544        # stored spatial freqs (32 * 17)
NFP = 640       # padded: 5 chunks of 128


# ----------------------------------------------------------------- constants
def build_constants(weight):
    c = np.arange(128)
    k = np.arange(128)
    ang = 2 * np.pi * np.outer(c, k) / 128.0
    fc_pack = np.concatenate([np.cos(ang), -np.sin(ang)], axis=1).astype(np.float32)

    f = np.arange(NFP)
    p = np.where(f < NF, f // 17, 0)
    q = np.where(f < NF, f % 17, 0)
    valid = (f < NF).astype(np.float64)

    # f2d [128, 8, 5, 384]: cols of block fb: [-sin | cos | +sin].
    # With PSUM blocks laid out [Xi | Xr], the A-pass (lhsT=X1r) reads
    # [-sin | cos] (cols 0:256) and the B-pass (lhsT=X1i) reads
    # [cos | sin] (cols 128:384) -- both contiguous, N=256 per block, so
    # fb-paired matmuls run at N=512 full fp32r rate.
    f2d = np.zeros((128, 8, 5, 384), dtype=np.float32)
    hw_p = np.arange(128)
    for t in range(8):
        h = 4 * t + hw_p // 32
        w = hw_p % 32
        for fb in range(5):
            sl = slice(fb * 128, (fb + 1) * 128)
            ang2 = 2 * np.pi * (np.outer(h, p[sl]) + np.outer(w, q[sl])) / 32.0
            f2d[:, t, fb, 0:128] = -np.sin(ang2) * valid[sl]
            f2d[:, t, fb, 128:256] = np.cos(ang2) * valid[sl]
            f2d[:, t, fb, 256:384] = np.sin(ang2) * valid[sl]

    kk = np.arange(128)
    j = np.arange(32)
    ang3 = 2 * np.pi * np.outer(kk, j) / 32.0
    er = np.cos(ang3)
    ei = np.sin(ang3)
    epack1 = np.concatenate([er, ei], axis=1).astype(np.float32)
    epack2 = np.concatenate([-ei, er], axis=1).astype(np.float32)

    k2d = np.zeros((128, 5, 2, 1024), dtype=np.float32)
    yz = np.arange(1024)
    y = yz // 32
    z = yz % 32
    for fc in range(5):
        sl = slice(fc * 128, (fc + 1) * 128)
        ang4 = 2 * np.pi * (np.outer(p[sl], y) + np.outer(q[sl], z)) / 32.0
        k2d[:, fc, 0, :] = np.cos(ang4) * valid[sl][:, None]
        k2d[:, fc, 1, :] = -np.sin(ang4) * valid[sl][:, None]

    w_hat = np.fft.rfftn(weight.astype(np.float64), s=(CIN, L, L), axes=(1, 2, 3))
    alpha = np.full(17, 2.0)
    alpha[0] = 1.0
    alpha[16] = 1.0
    w_hat = w_hat * alpha[None, None, None, :] / (128.0 * 32.0 * 32.0)
    wr = np.zeros((128, NFIL, NFP), dtype=np.float32)
    wi = np.zeros((128, NFIL, NFP), dtype=np.float32)
    wr[:, :, :NF] = np.transpose(w_hat.real, (1, 0, 2, 3)).reshape(128, NFIL, NF)
    wi[:, :, :NF] = np.transpose(w_hat.imag, (1, 0, 2, 3)).reshape(128, NFIL, NF)
    return {
        "fc_pack": fc_pack,
        "f2d": f2d,
        "epack1": epack1,
        "epack2": epack2,
        "k2d": k2d,
        "wr": wr,
        "wi": wi,
    }


# ----------------------------------------------------------------- program
def build_program(dbg=False):
    nc = bacc.Bacc("TRN2", target_bir_lowering=False, debug=False)
    x_d = nc.dram_tensor("x", [B_LOC, 128, 32, 32], F32R, kind="ExternalInput")
    fc_d = nc.dram_tensor("fc_pack", [128, 256], F32R, kind="ExternalInput")
    f2d_d = nc.dram_tensor("f2d", [128, 8, 5, 384], F32R, kind="ExternalInput")
    e1_d = nc.dram_tensor("epack1", [128, 64], F32R, kind="ExternalInput")
    e2_d = nc.dram_tensor("epack2", [128, 64], F32R, kind="ExternalInput")
    k2d_d = nc.dram_tensor("k2d", [128, 5, 2, 1024], F32R, kind="ExternalInput")
    wr_d = nc.dram_tensor("wr", [128, NFIL, NFP], F32R, kind="ExternalInput")
    wi_d = nc.dram_tensor("wi", [128, NFIL, NFP], F32R, kind="ExternalInput")
    out_d = nc.dram_tensor("out", [B_LOC, 256, 32, 32], F32, kind="ExternalOutput")
    if dbg:
        dbg_x1 = nc.dram_tensor("dbg_x1", [128, 8, 2, 128], F32R, kind="ExternalOutput")
        dbg_xf = nc.dram_tensor("dbg_xf", [128, 2, NFP], F32R, kind="ExternalOutput")
        dbg_a = nc.dram_tensor("dbg_a", [128, 5, 2, NFIL, 32], F32R, kind="ExternalOutput")

    with tile.TileContext(nc) as tc:
        with (
            tc.tile_pool(name="consts", bufs=1) as cpool,
            tc.tile_pool(name="xin", bufs=2) as xpool,
            tc.tile_pool(name="x1", bufs=2) as x1pool,
            tc.tile_pool(name="xf", bufs=2) as xfpool,
            tc.tile_pool(name="z", bufs=2) as zpool,
            tc.tile_pool(name="a", bufs=1) as apool,
            tc.tile_pool(name="o", bufs=2) as opool,
            tc.tile_pool(name="ps1", bufs=1, space="PSUM") as ps1pool,
            tc.tile_pool(name="ps2", bufs=3, space="PSUM") as ps2pool,
            tc.tile_pool(name="psi1", bufs=2, space="PSUM") as psi1pool,
            tc.tile_pool(name="psi2", bufs=2, space="PSUM") as psi2pool,
        ):
            # ---- load constants; ordered + chunked so early compute
            # stages never wait on late-needed constants.
            fc_sb = cpool.tile([128, 256], F32R)
            nc.sync.dma_start(out=fc_sb[:], in_=fc_d[:])
            e1_sb = cpool.tile([128, 64], F32R)
            nc.sync.dma_start(out=e1_sb[:], in_=e1_d[:])
            e2_sb = cpool.tile([128, 64], F32R)
            nc.sync.dma_start(out=e2_sb[:], in_=e2_d[:])
            f2d_sb = cpool.tile([128, 8, 5, 384], F32R)
            for t in range(8):
                nc.sync.dma_start(out=f2d_sb[:, t], in_=f2d_d[:, t])
            wr_sb = cpool.tile([128, NFIL, NFP], F32R)
            nc.sync.dma_start(out=wr_sb[:], in_=wr_d[:])
            wi_sb = cpool.tile([128, NFIL, NFP], F32R)
            nc.sync.dma_start(out=wi_sb[:], in_=wi_d[:])
            k2d_sb = cpool.tile([128, 5, 2, 1024], F32R)
            for fc in range(5):
                nc.sync.dma_start(out=k2d_sb[:, fc], in_=k2d_d[:, fc])

            for b in range(B_LOC):
                # ---- load x[b]: [c=128, h, w]
                xt = xpool.tile([128, 32, 32], F32R, tag="xt")
                nc.scalar.dma_start(out=xt[:], in_=x_d[b])

                # ---- S1: channel DFT; x1 slots: 0=X1r, 1=X1i
                x1 = x1pool.tile([128, 8, 2, 128], F32R, tag="x1")
                for t in range(8):
                    ps = ps1pool.tile([128, 256], F32, tag="ps1")
                    nc.tensor.matmul(
                        ps[:], xt[:, 4 * t:4 * t + 4, :], fc_sb[:],
                        start=True, stop=True,
                    )
                    nc.vector.tensor_copy(x1[:, t, 0, :], ps[:, 0:128])
                    nc.scalar.copy(x1[:, t, 1, :], ps[:, 128:256])

                if dbg and b == 0:
                    nc.sync.dma_start(out=dbg_x1[:], in_=x1[:])

                # ---- S2: joint 2D DFT, complex combine via PSUM accumulation.
                # psum block fb: cols [Xr(128) | Xi(128)]
                # Xr = X1r@cos + X1i@sin ;  Xi = X1i@cos - X1r@sin
                s2ps = [
                    ps2pool.tile([128, 512], F32, tag="ps2", name=f"s2ps{b}_{g}")
                    for g in range(3)
                ]

                def s2slot(fb):
                    return s2ps[fb // 2][:, (fb % 2) * 256:(fb % 2) * 256 + 256]

                # One accumulation group per PSUM bank: the A-pass spans the
                # whole bank (fb-pair) and carries the only start=True.
                # PSUM block fb = [Xi(128) | Xr(128)]:
                #   A (lhsT=X1r) @ [-sin | cos] ;  B (lhsT=X1i) @ [cos | sin]
                for t in range(8):
                    for g in range(3):
                        slot = s2ps[g]
                        width = 512 if g < 2 else 256
                        pair = slice(2 * g, 2 * g + 2) if g < 2 else slice(4, 5)
                        nc.tensor.matmul(
                            slot[:, 0:width], x1[:, t, 0, :],
                            f2d_sb[:, t, pair, 0:256],
                            start=(t == 0), stop=False,
                            skip_group_check=True,
                        )
                    for g in range(3):
                        slot = s2ps[g]
                        width = 512 if g < 2 else 256
                        pair = slice(2 * g, 2 * g + 2) if g < 2 else slice(4, 5)
                        nc.tensor.matmul(
                            slot[:, 0:width], x1[:, t, 1, :],
                            f2d_sb[:, t, pair, 128:384],
                            start=False, stop=(t == 7),
                            skip_group_check=True,
                        )

                # copy PSUM -> SBUF, de-interleaving to contiguous
                # xr_c / xi_c [128, 640] (f-major) for fast 1D DVE ops
                xr_c = xfpool.tile([128, NFP], F32R, tag="xr")
                xi_c = xfpool.tile([128, NFP], F32R, tag="xi")
                for g in range(3):
                    slot = s2ps[g]
                    if g < 2:
                        sv = slot[:].rearrange("p (a c) -> p a c", a=2)
                        nc.vector.tensor_copy(
                            xi_c[:, g * 256:(g + 1) * 256], sv[:, :, 0:128])
                        nc.vector.tensor_copy(
                            xr_c[:, g * 256:(g + 1) * 256], sv[:, :, 128:256])
                    else:
                        nc.vector.tensor_copy(xi_c[:, 512:640], slot[:, 0:128])
                        nc.vector.tensor_copy(xr_c[:, 512:640], slot[:, 128:256])

                if dbg and b == 0:
                    nc.sync.dma_start(out=dbg_xf[:, 0], in_=xr_c[:])
                    nc.sync.dma_start(out=dbg_xf[:, 1], in_=xi_c[:])

                # ---- per-n: complex multiply (DVE) + I1 matmuls
                a_sb = apool.tile([128, 5, 2, NFIL, 32], F32R, tag="a")
                for n in range(8):
                    zr = zpool.tile([128, NFP], F32R, tag="zr")
                    zi = zpool.tile([128, NFP], F32R, tag="zi")
                    t1 = zpool.tile([128, NFP], F32R, tag="t1", bufs=1)
                    t2 = zpool.tile([128, NFP], F32R, tag="t2", bufs=1)
                    nc.vector.tensor_mul(t1[:], xr_c[:], wr_sb[:, n, :])
                    nc.vector.tensor_mul(t2[:], xi_c[:], wi_sb[:, n, :])
                    nc.vector.tensor_sub(zr[:], t1[:], t2[:])
                    nc.vector.tensor_mul(t1[:], xr_c[:], wi_sb[:, n, :])
                    nc.vector.tensor_mul(t2[:], xi_c[:], wr_sb[:, n, :])
                    nc.vector.tensor_add(zi[:], t1[:], t2[:])

                    ips = psi1pool.tile([128, 320], F32, tag="psi1")
                    for fc in range(5):
                        col = slice(fc * 64, (fc + 1) * 64)
                        zsl = slice(fc * 128, (fc + 1) * 128)
                        nc.tensor.matmul(
                            ips[:, col], zr[:, zsl], e1_sb[:],
                            start=True, stop=False,
                        )
                        nc.tensor.matmul(
                            ips[:, col], zi[:, zsl], e2_sb[:],
                            start=False, stop=True,
                        )
                    # scatter [fc, comp, j] cols of ips into a_sb[:, fc, comp, n, :]
                    nc.scalar.copy(
                        a_sb[:, :, :, n, :],
                        ips[:].rearrange("p (fc c j) -> p fc c j", fc=5, c=2),
                    )

                if dbg and b == 0:
                    nc.sync.dma_start(out=dbg_a[:], in_=a_sb[:])

                # ---- I2: joint 2D inverse + Re extraction
                for mh in range(2):
                    for nzc in range(2):
                        ops = psi2pool.tile([128, 512], F32, tag="psi2")
                        for fc in range(5):
                            for comp in range(2):
                                lhsT = a_sb[:, fc, comp,
                                            mh * 4:(mh + 1) * 4, :]
                                rhs = k2d_sb[:, fc, comp,
                                             nzc * 512:(nzc + 1) * 512]
                                nc.tensor.matmul(
                                    ops[:], lhsT, rhs,
                                    start=(fc == 0 and comp == 0),
                                    stop=(fc == 4 and comp == 1),
                                )
                        o_sb = opool.tile([128, 512], F32, tag="o")
                        nc.vector.tensor_copy(o_sb[:], ops[:])
                        dst = out_d[b, mh * 128:(mh + 1) * 128].rearrange(
                            "c h w -> c (h w)")[:, nzc * 512:(nzc + 1) * 512]
                        nc.scalar.dma_start(out=dst, in_=o_sb[:])
    nc.compile()
    return nc


_CACHE = {}


def kernel(x, weight):
    x = np.ascontiguousarray(np.asarray(x, dtype=np.float32))
    weight = np.asarray(weight, dtype=np.float32)
    consts = build_constants(weight)
    if "nc" not in _CACHE:
        _CACHE["nc"] = build_program()
    nc = _CACHE["nc"]
    in_maps = []
    for i in range(N_CORES):
        m = {"x": x[i * B_LOC:(i + 1) * B_LOC]}
        m.update(consts)
        in_maps.append(m)
    res = run_bass_kernel_spmd(nc, in_maps, core_ids=list(range(N_CORES)))
    out = np.concatenate([r["out"] for r in res.results], axis=0)
    return out


if __name__ == "__main__":
    import jax

    sys.path.insert(0, "/root/problem")
    from reference import setup_inputs, reference

    with jax.default_device(jax.devices("cpu")[0]):
        inputs = setup_inputs()
        inputs = {k: np.asarray(v) for k, v in inputs.items()}
        expected = np.asarray(reference(**inputs))
    actual = kernel(**inputs)
    err = np.linalg.norm(actual - expected) / np.linalg.norm(expected)
    print("Relative error:", err)
